# revision 11
# baseline (speedup 1.0000x reference)
"""GCN encoder (7-layer GCNConv) on 8 Trainium2 NeuronCores.

Strategy (node-sharded, SPMD):
  - Nodes are permuted and balanced into 8 cores x 10 target-tiles of 128
    slots each (degree-balanced bins so every tile has <= 2176 incoming
    edges = 17 edge-tiles of 128).
  - Per layer l: z = h @ W_l computed locally (dense, bf16 PE matmuls with
    activations as the stationary operand so output is node-major), z is
    AllGathered to every core (bf16), then per target-tile the incoming
    edge messages are fetched with dma_gather (per-edge row gather from
    the gathered z) and segment-summed on the TensorEngine by multiplying
    with a per-tile sparse indicator matrix S (S[e, t] = gcn_norm of edge
    e into target t).  Bias is folded in as one extra matmul; ReLU+cast on
    the Scalar engine.  h -> h^T for the next dense layer is done with an
    SBUF-source transposing dma_gather.
  - gcn_norm / edge bucketing / permutation are host-side preprocessing;
    all FLOPs (dense transforms + message aggregation) run on device.
"""

import os
import sys
import types

sys.path.insert(0, "/opt/trn_rl_repo")

import numpy as np
import ml_dtypes

NCORES = 8
N = 10000
E = 160000
DIN = 128
DH = 1000
DOUT = 256

TPC = 10  # target tiles (groups) per core
NP_ = TPC * 128  # 1280 node slots per core
NTOT = NCORES * NP_  # 10240
KT = 17  # edge tiles per group
EPG = KT * 128  # 2176 edge slots per group
NLP = 1024  # padded hidden width

BF = ml_dtypes.bfloat16

LAYER_NL = [NLP] * 6 + [DOUT]
LAYER_KL = [1] + [8] * 6

_CACHE = {}

LAST_EXEC_NS = None


def _install_ntff_shim():
    try:
        import antenv

        if hasattr(antenv, "axon_hooks"):
            return
        from trn_agent_boot.trn_boot import _ntff_profile_via_ctypes

        hook = _ntff_profile_via_ctypes("/opt/axon/libaxon_pjrt.so")
        mod = types.ModuleType("antenv.axon_hooks")
        mod.get_axon_ntff_profile_hook = lambda: hook
        mod.set_axon_ntff_profile_hook = lambda h: None
        sys.modules["antenv.axon_hooks"] = mod
        antenv.axon_hooks = mod
    except Exception:
        pass


def _wrap_idx(idx):
    """[n] int -> [128, n/16] int16 (i -> row i%16, col i//16), 8x replicated."""
    n = idx.shape[0]
    w = np.asarray(idx, np.int16).reshape(n // 16, 16).T
    return np.tile(w, (8, 1))


def _build_bass():
    import concourse.mybir as mybir
    from concourse import bacc, tile

    SKIP_GATHER = os.environ.get("GCN_SKIP_GATHER") == "1"
    SKIP_COLL = os.environ.get("GCN_SKIP_COLL") == "1"
    SKIP_TRANS = os.environ.get("GCN_SKIP_TRANS") == "1"

    f32 = mybir.dt.float32
    b16 = mybir.dt.bfloat16
    i16 = mybir.dt.int16
    RG = [list(range(NCORES))]

    nc = bacc.Bacc("TRN2", target_bir_lowering=False, debug=False, num_devices=NCORES)

    xT_d = nc.dram_tensor("xT", [128, NP_], b16, kind="ExternalInput").ap()
    w_d = [
        nc.dram_tensor(f"w{l}", [128, LAYER_KL[l], LAYER_NL[l]], b16, kind="ExternalInput").ap()
        for l in range(7)
    ]
    bias_d = [
        nc.dram_tensor(f"bias{l}", [128, LAYER_NL[l]], b16, kind="ExternalInput").ap()
        for l in range(7)
    ]
    ones_d = nc.dram_tensor("ones", [128, 128], b16, kind="ExternalInput").ap()
    s_d = nc.dram_tensor("s", [128, TPC, KT, 128], b16, kind="ExternalInput").ap()
    eidx_d = nc.dram_tensor("eidx", [128, TPC, EPG // 16], i16, kind="ExternalInput").ap()
    tidx_d = nc.dram_tensor("tidx", [128, NP_ // 16], i16, kind="ExternalInput").ap()
    out_d = nc.dram_tensor("out", [NP_, DOUT], f32, kind="ExternalOutput").ap()
    out_v = out_d.rearrange("(g p) f -> p g f", p=128)

    with tile.TileContext(nc) as tc:
        with (
            tc.tile_pool(name="const", bufs=1) as cpool,
            tc.tile_pool(name="w", bufs=2) as wpool,
            tc.tile_pool(name="h", bufs=1) as hpool,
            tc.tile_pool(name="ht", bufs=1) as htpool,
            tc.tile_pool(name="z", bufs=3) as zpool,
            tc.tile_pool(name="m", bufs=2) as mpool,
            tc.tile_pool(name="o", bufs=2) as opool,
            tc.tile_pool(name="psD", bufs=2, space="PSUM") as psD,
            tc.tile_pool(name="psA", bufs=2, space="PSUM") as psA,
            tc.tile_pool(name="dram", bufs=2, space="DRAM") as dpool,
        ):
            xT_sb = cpool.tile([128, NP_], b16)
            nc.sync.dma_start(xT_sb[:], xT_d[:])
            ones_sb = cpool.tile([128, 128], b16)
            nc.sync.dma_start(ones_sb[:], ones_d[:])
            s_sb = cpool.tile([128, TPC, KT, 128], b16)
            nc.sync.dma_start(s_sb[:], s_d[:])
            eidx_sb = cpool.tile([128, TPC, EPG // 16], i16)
            nc.sync.dma_start(eidx_sb[:], eidx_d[:])
            tidx_sb = cpool.tile([128, NP_ // 16], i16)
            nc.sync.dma_start(tidx_sb[:], tidx_d[:])
            bias_sb = []
            for l in range(7):
                b_sb = cpool.tile([128, LAYER_NL[l]], b16, name=f"bias_sb{l}")
                nc.sync.dma_start(b_sb[:], bias_d[l][:])
                bias_sb.append(b_sb)

            hT_prev = None
            for l in range(7):
                NL = LAYER_NL[l]
                KL = LAYER_KL[l]
                nch = 2 if NL == NLP else 1
                fcw = 512 if NL == NLP else NL

                w_sb = wpool.tile([128, KL, NL], b16, tag="w", name=f"w_sb{l}")
                nc.sync.dma_start(w_sb[:], w_d[l][:])
                zb = dpool.tile([NP_, NL], b16, tag="zb", name=f"zb{l}")
                zf = dpool.tile(
                    [NTOT, NL], b16, addr_space="Shared", tag="zf", name=f"zf{l}"
                )

                # dense: z = h @ W  (node-major out; activations are lhsT)
                for n in range(nch):
                    for m in range(TPC):
                        zp = psD.tile([128, fcw], f32, tag="psD", name=f"zp{l}_{n}_{m}")
                        for k in range(KL):
                            if l == 0:
                                lhsT = xT_sb[:, m * 128 : (m + 1) * 128]
                            else:
                                lhsT = hT_prev[
                                    :, m // 2, k, (m % 2) * 128 : (m % 2) * 128 + 128
                                ]
                            nc.tensor.matmul(
                                zp[:],
                                lhsT,
                                w_sb[:, k, n * 512 : n * 512 + fcw],
                                start=(k == 0),
                                stop=(k == KL - 1),
                            )
                        z_sb = zpool.tile([128, fcw], b16, tag="z", name=f"z{l}_{n}_{m}")
                        nc.vector.tensor_copy(z_sb[:], zp[:])
                        nc.sync.dma_start(
                            zb[m * 128 : (m + 1) * 128, n * 512 : n * 512 + fcw],
                            z_sb[:],
                        )

                if SKIP_COLL:
                    nc.sync.dma_start(zf[0:NP_, :], zb[:])
                else:
                    nc.gpsimd.collective_compute(
                        "AllGather",
                        mybir.AluOpType.bypass,
                        replica_groups=RG,
                        ins=[zb[:].opt()],
                        outs=[zf[:].opt()],
                    )

                if l < 6:
                    h_sb = hpool.tile([128, TPC, NLP], b16, tag="h", name=f"h{l}")

                # aggregation: for each target tile, gather messages + S matmul
                for g in range(TPC):
                    for n in range(nch):
                        ap = psA.tile([128, fcw], f32, tag="psA", name=f"ap{l}_{g}_{n}")
                        if not SKIP_GATHER:
                            msgs = mpool.tile(
                                [128, KT, fcw], b16, tag="m", name=f"msgs{l}_{g}_{n}"
                            )
                            # HW limit: dma_gather faults above ~1024 idxs/call
                            off = 0
                            while off < EPG:
                                c = min(1024, EPG - off)
                                nc.gpsimd.dma_gather(
                                    msgs[:, off // 128 : (off + c) // 128, :],
                                    zf[:, n * 512 : n * 512 + fcw],
                                    eidx_sb[:, g, off // 16 : (off + c) // 16],
                                    num_idxs=c,
                                    num_idxs_reg=c,
                                    elem_size=fcw,
                                    elem_step=NL,
                                )
                                off += c
                            for k in range(KT):
                                nc.tensor.matmul(
                                    ap[:],
                                    s_sb[:, g, k, :],
                                    msgs[:, k, :],
                                    start=(k == 0),
                                    stop=False,
                                )
                        nc.tensor.matmul(
                            ap[:],
                            ones_sb[:],
                            bias_sb[l][:, n * 512 : n * 512 + fcw],
                            start=SKIP_GATHER,
                            stop=True,
                        )
                        if l < 6:
                            nc.scalar.activation(
                                h_sb[:, g, n * 512 : n * 512 + fcw],
                                ap[:],
                                mybir.ActivationFunctionType.Relu,
                            )
                        else:
                            o_sb = opool.tile([128, DOUT], f32, tag="o", name=f"o{g}")
                            nc.scalar.activation(
                                o_sb[:], ap[:], mybir.ActivationFunctionType.Copy
                            )
                            nc.sync.dma_start(out_v[:, g, :], o_sb[:])

                if l < 6:
                    # hT layout: [128, chunk(5), k(8), 256]; chunk ci holds
                    # nodes ci*256..ci*256+255 (m-tiles 2ci, 2ci+1)
                    hT = htpool.tile([128, 5, 8, 256], b16, tag="ht", name=f"hT{l}")
                    if SKIP_TRANS:
                        nc.vector.tensor_copy(
                            hT.rearrange("p a b c -> p (a b c)"),
                            h_sb.rearrange("p a b -> p (a b)"),
                        )
                    else:
                        for ci in range(5):
                            nc.gpsimd.dma_gather(
                                hT[:, ci, :, :],
                                h_sb[:],
                                tidx_sb[:, ci * 16 : (ci + 1) * 16],
                                num_idxs=256,
                                num_idxs_reg=256,
                                elem_size=NLP,
                                transpose=True,
                                sbuf_tokens_per_rank=128,
                                sbuf_free_dim_per_rank=NLP * 2,
                            )
                    hT_prev = hT

    nc.compile()
    return nc


def _preprocess(x, edge_index, edge_weight):
    """gcn_norm + node permutation + per-core edge buckets (host side)."""
    ei = np.asarray(edge_index)
    row = np.concatenate([ei[0], np.arange(N)]).astype(np.int64)
    col = np.concatenate([ei[1], np.arange(N)]).astype(np.int64)
    w = np.concatenate(
        [np.asarray(edge_weight, np.float64), np.ones(N, np.float64)]
    )
    deg = np.zeros(N, np.float64)
    np.add.at(deg, col, w)
    dis = np.where(deg > 0, 1.0 / np.sqrt(deg), 0.0)
    norm = (dis[row] * w * dis[col]).astype(np.float32)

    # balance nodes into 80 bins (cap 128 nodes) by in-degree
    indeg = np.bincount(col, minlength=N)
    NB = NCORES * TPC
    order = np.argsort(-indeg, kind="stable")
    load = np.zeros(NB, np.int64)
    cnt = np.zeros(NB, np.int64)
    binof = np.empty(N, np.int64)
    slotof = np.empty(N, np.int64)
    for v in order:
        feas = np.flatnonzero(cnt < 128)
        b = feas[np.argmin(load[feas])]
        binof[v] = b
        slotof[v] = cnt[b]
        cnt[b] += 1
        load[b] += indeg[v]
    assert load.max() <= EPG, f"bin overflow: {load.max()} > {EPG}"
    core = binof // TPC
    grp = binof % TPC
    pid = core * NP_ + grp * 128 + slotof  # permuted global id

    # bucket edges by target bin, assign sequential slots
    ebin = binof[col]
    eorder = np.argsort(ebin, kind="stable")
    ebin_s = ebin[eorder]
    counts = np.bincount(ebin_s, minlength=NB)
    starts = np.concatenate([[0], np.cumsum(counts)[:-1]])
    eslot = np.arange(len(eorder)) - starts[ebin_s]
    ec = ebin_s // TPC
    eg = ebin_s % TPC
    ek = eslot // 128
    ep = eslot % 128
    et = slotof[col[eorder]]
    S = np.zeros((NCORES, 128, TPC, KT, 128), np.float32)
    S[ec, ep, eg, ek, et] = norm[eorder]
    IDX = np.zeros((NCORES, TPC, EPG), np.int64)
    IDX[ec, eg, eslot] = pid[row[eorder]]
    return pid, S, IDX


def kernel(x, edge_index, edge_weight, W1, b1, Wmid, bmid, W7, b7):
    global LAST_EXEC_NS
    trace = os.environ.get("GCN_TRACE") == "1"
    if trace:
        _install_ntff_shim()

    from concourse import bass_utils

    x = np.asarray(x, np.float32)
    pid, S, IDX = _preprocess(x, edge_index, edge_weight)

    # per-core x^T (bf16), empty slots zero
    xT = np.zeros((NCORES, 128, NP_), np.float32)
    xT[pid // NP_, :, pid % NP_] = x
    xT = xT.astype(BF)

    # weights / biases, padded + k-striped, bf16
    def kstripe(W, KL, NL):
        Wp = np.zeros((KL * 128, NL), np.float32)
        Wp[: W.shape[0], : W.shape[1]] = np.asarray(W, np.float32)
        return Wp.reshape(KL, 128, NL).transpose(1, 0, 2).astype(BF)

    Ws = [kstripe(np.asarray(W1), 1, NLP)]
    for i in range(5):
        Ws.append(kstripe(np.asarray(Wmid)[i], 8, NLP))
    Ws.append(kstripe(np.asarray(W7), 8, DOUT))
    bs = []
    for i, b in enumerate([b1] + [np.asarray(bmid)[i] for i in range(5)] + [b7]):
        NL = LAYER_NL[i]
        bp = np.zeros(NL, np.float32)
        bp[: b.shape[0]] = np.asarray(b, np.float32)
        bs.append(np.broadcast_to(bp.astype(BF), (128, NL)).copy())

    ones = np.full((128, 128), 1.0 / 128.0, np.float32).astype(BF)
    tidx = _wrap_idx(np.arange(NP_))

    if "nc" not in _CACHE:
        _CACHE["nc"] = _build_bass()
    nc = _CACHE["nc"]

    in_maps = []
    for c in range(NCORES):
        eidx_c = np.stack(
            [_wrap_idx(IDX[c, g]) for g in range(TPC)], axis=1
        )  # [128, TPC, 136]
        m = {
            "xT": xT[c],
            "ones": ones,
            "s": np.ascontiguousarray(S[c].astype(BF)),
            "eidx": np.ascontiguousarray(eidx_c),
            "tidx": tidx,
        }
        for l in range(7):
            m[f"w{l}"] = Ws[l]
            m[f"bias{l}"] = bs[l]
        in_maps.append(m)

    res = bass_utils.run_bass_kernel_spmd(
        nc, in_maps, core_ids=list(range(NCORES)), trace=trace
    )
    if trace:
        LAST_EXEC_NS = res.exec_time_ns
        print(f"HW exec time: {res.exec_time_ns} ns")
        if res.instructions_and_trace is not None:
            print(f"trace: {res.instructions_and_trace[1]}")

    out_full = np.empty((N, DOUT), np.float32)
    percore = np.stack([res.results[c]["out"] for c in range(NCORES)])  # [8,1280,256]
    out_full = percore[pid // NP_, pid % NP_]
    return out_full


# revision 14
# speedup vs baseline: 1.4712x; 1.4712x over previous
"""GCN encoder (7-layer GCNConv) on 8 Trainium2 NeuronCores.

Strategy (node-sharded, SPMD):
  - Nodes are permuted and balanced into 8 cores x 10 target-tiles of 128
    slots each (degree-balanced bins so every tile has <= 2176 incoming
    edges = 17 edge-tiles of 128).
  - Per layer l: z = h @ W_l computed locally (dense, bf16 PE matmuls with
    activations as the stationary operand so output is node-major), z is
    AllGathered to every core (bf16, split into two column-halves so the
    second half overlaps the first half's gathers), then per target-tile
    the incoming edge messages are fetched with dma_gather (per-edge row
    gather from the gathered z, 4 SWDGE queues for parallel descriptor
    emission) and segment-summed on the TensorEngine by multiplying with a
    per-tile sparse indicator matrix S (S[e, t] = gcn_norm of edge e into
    target t).  Bias is folded in as one extra matmul; ReLU+cast on the
    Scalar engine.  h -> h^T for the next dense layer is done with
    SBUF-source transposing dma_gathers, chunked per 256 nodes so the next
    layer's dense can start before the whole layer finishes.
  - gcn_norm / edge bucketing / permutation are host-side preprocessing;
    all FLOPs (dense transforms + message aggregation) run on device.
"""

import os
import sys
import types

sys.path.insert(0, "/opt/trn_rl_repo")

import numpy as np
import ml_dtypes

NCORES = 8
N = 10000
E = 160000
DIN = 128
DH = 1000
DOUT = 256

TPC = 10  # target tiles (groups) per core
NP_ = TPC * 128  # 1280 node slots per core
NTOT = NCORES * NP_  # 10240
KT = 17  # edge tiles per group
EPG = KT * 128  # 2176 edge slots per group
NLP = 1024  # padded hidden width
NQ = 4  # SWDGE queues

BF = ml_dtypes.bfloat16

LAYER_NL = [NLP] * 6 + [DOUT]
LAYER_KL = [1] + [8] * 6

_CACHE = {}

LAST_EXEC_NS = None
LAST_TRACE = None


def _install_ntff_shim():
    try:
        import antenv

        if hasattr(antenv, "axon_hooks"):
            return
        from trn_agent_boot.trn_boot import _ntff_profile_via_ctypes

        hook = _ntff_profile_via_ctypes("/opt/axon/libaxon_pjrt.so")
        mod = types.ModuleType("antenv.axon_hooks")
        mod.get_axon_ntff_profile_hook = lambda: hook
        mod.set_axon_ntff_profile_hook = lambda h: None
        sys.modules["antenv.axon_hooks"] = mod
        antenv.axon_hooks = mod
    except Exception:
        pass


def _wrap_idx(idx):
    """[n] int -> [128, n/16] int16 (i -> row i%16, col i//16), 8x replicated."""
    n = idx.shape[0]
    w = np.asarray(idx, np.int16).reshape(n // 16, 16).T
    return np.tile(w, (8, 1))


def _build_bass():
    import concourse.mybir as mybir
    from concourse import bacc, tile

    f32 = mybir.dt.float32
    b16 = mybir.dt.bfloat16
    i16 = mybir.dt.int16
    RG = [list(range(NCORES))]

    nc = bacc.Bacc(
        "TRN2",
        target_bir_lowering=False,
        debug=False,
        num_devices=NCORES,
        num_swdge_queues=NQ,
    )

    xT_d = nc.dram_tensor("xT", [128, NP_], b16, kind="ExternalInput").ap()
    w_d = [
        nc.dram_tensor(
            f"w{l}", [128, LAYER_KL[l], LAYER_NL[l]], b16, kind="ExternalInput"
        ).ap()
        for l in range(7)
    ]
    bias_d = [
        nc.dram_tensor(f"bias{l}", [128, LAYER_NL[l]], b16, kind="ExternalInput").ap()
        for l in range(7)
    ]
    ones_d = nc.dram_tensor("ones", [128, 128], b16, kind="ExternalInput").ap()
    s_d = nc.dram_tensor("s", [128, TPC, KT, 128], b16, kind="ExternalInput").ap()
    eidx_d = nc.dram_tensor(
        "eidx", [128, TPC, EPG // 16], i16, kind="ExternalInput"
    ).ap()
    tidx_d = nc.dram_tensor("tidx", [128, 16], i16, kind="ExternalInput").ap()
    out_d = nc.dram_tensor("out", [NP_, DOUT], f32, kind="ExternalOutput").ap()
    out_v = out_d.rearrange("(g p) f -> p g f", p=128)

    qctr = [0]

    def next_q():
        q = qctr[0] % NQ
        qctr[0] += 1
        return q

    with tile.TileContext(nc) as tc:
        with (
            tc.tile_pool(name="const", bufs=1) as cpool,
            tc.tile_pool(name="w", bufs=2) as wpool,
            tc.tile_pool(name="h", bufs=1) as hpool,
            tc.tile_pool(name="ht", bufs=1) as htpool,
            tc.tile_pool(name="z", bufs=3) as zpool,
            tc.tile_pool(name="m", bufs=2) as mpool,
            tc.tile_pool(name="o", bufs=2) as opool,
            tc.tile_pool(name="psD", bufs=2, space="PSUM") as psD,
            tc.tile_pool(name="psA", bufs=3, space="PSUM") as psA,
            tc.tile_pool(name="dram", bufs=2, space="DRAM") as dpool,
        ):
            xT_sb = cpool.tile([128, NP_], b16)
            nc.sync.dma_start(xT_sb[:], xT_d[:])
            ones_sb = cpool.tile([128, 128], b16)
            nc.sync.dma_start(ones_sb[:], ones_d[:])
            s_sb = cpool.tile([128, TPC, KT, 128], b16)
            nc.sync.dma_start(s_sb[:], s_d[:])
            eidx_sb = cpool.tile([128, TPC, EPG // 16], i16)
            nc.sync.dma_start(eidx_sb[:], eidx_d[:])
            tidx_sb = cpool.tile([128, 16], i16)
            nc.sync.dma_start(tidx_sb[:], tidx_d[:])
            bias_sb = []
            for l in range(7):
                b_sb = cpool.tile([128, LAYER_NL[l]], b16, name=f"bias_sb{l}")
                nc.sync.dma_start(b_sb[:], bias_d[l][:])
                bias_sb.append(b_sb)

            # h / hT in 5 chunks of 256 nodes for fine-grained pipelining
            hT_prev = None
            for l in range(7):
                NL = LAYER_NL[l]
                KL = LAYER_KL[l]
                nch = 2 if NL == NLP else 1
                fcw = 512 if NL == NLP else NL

                w_sb = wpool.tile([128, KL, NL], b16, tag="w", name=f"w_sb{l}")
                nc.sync.dma_start(w_sb[:], w_d[l][:])

                # column-split bounce + gathered buffers (n-th half)
                zbs = [
                    dpool.tile([NP_, fcw], b16, tag=f"zb{n}", name=f"zb{l}_{n}")
                    for n in range(nch)
                ]
                zfs = [
                    dpool.tile(
                        [NTOT, fcw],
                        b16,
                        addr_space="Shared",
                        tag=f"zf{n}",
                        name=f"zf{l}_{n}",
                    )
                    for n in range(nch)
                ]

                # dense: z = h @ W  (node-major out; activations are lhsT)
                for n in range(nch):
                    for m in range(TPC):
                        zp = psD.tile([128, fcw], f32, tag="psD", name=f"zp{l}_{n}_{m}")
                        for k in range(KL):
                            if l == 0:
                                lhsT = xT_sb[:, m * 128 : (m + 1) * 128]
                            else:
                                lhsT = hT_prev[m // 2][
                                    :, k, (m % 2) * 128 : (m % 2) * 128 + 128
                                ]
                            nc.tensor.matmul(
                                zp[:],
                                lhsT,
                                w_sb[:, k, n * 512 : n * 512 + fcw],
                                start=(k == 0),
                                stop=(k == KL - 1),
                            )
                        z_sb = zpool.tile(
                            [128, fcw], b16, tag="z", name=f"z{l}_{n}_{m}"
                        )
                        nc.vector.tensor_copy(z_sb[:], zp[:])
                        nc.sync.dma_start(
                            zbs[n][m * 128 : (m + 1) * 128, :], z_sb[:]
                        )
                    # AllGather this column half as soon as it is complete
                    nc.gpsimd.collective_compute(
                        "AllGather",
                        mybir.AluOpType.bypass,
                        replica_groups=RG,
                        ins=[zbs[n][:].opt()],
                        outs=[zfs[n][:].opt()],
                    )

                if l < 6:
                    h_c = [
                        hpool.tile(
                            [128, 2, NLP], b16, tag=f"h{ci}", name=f"h{l}_c{ci}"
                        )
                        for ci in range(5)
                    ]
                    hT_c = [
                        htpool.tile(
                            [128, 8, 256], b16, tag=f"ht{ci}", name=f"hT{l}_c{ci}"
                        )
                        for ci in range(5)
                    ]

                # aggregation: per target tile, gather messages + S matmuls
                for g in range(TPC):
                    for n in range(nch):
                        msgs = mpool.tile(
                            [128, KT, fcw], b16, tag="m", name=f"msgs{l}_{g}_{n}"
                        )
                        # HW limit: dma_gather faults above ~1024 idxs/call
                        off = 0
                        while off < EPG:
                            c = min(1024, EPG - off)
                            nc.gpsimd.dma_gather(
                                msgs[:, off // 128 : (off + c) // 128, :],
                                zfs[n][:],
                                eidx_sb[:, g, off // 16 : (off + c) // 16],
                                num_idxs=c,
                                num_idxs_reg=c,
                                elem_size=fcw,
                                elem_step=fcw,
                                queue_num=next_q(),
                            )
                            off += c
                        ap = psA.tile(
                            [128, fcw], f32, tag="psA", name=f"ap{l}_{g}_{n}"
                        )
                        for k in range(KT):
                            nc.tensor.matmul(
                                ap[:],
                                s_sb[:, g, k, :],
                                msgs[:, k, :],
                                start=(k == 0),
                                stop=False,
                            )
                        nc.tensor.matmul(
                            ap[:],
                            ones_sb[:],
                            bias_sb[l][:, n * 512 : n * 512 + fcw],
                            start=False,
                            stop=True,
                        )
                        if l < 6:
                            nc.scalar.activation(
                                h_c[g // 2][:, g % 2, n * 512 : n * 512 + fcw],
                                ap[:],
                                mybir.ActivationFunctionType.Relu,
                            )
                        else:
                            o_sb = opool.tile([128, DOUT], f32, tag="o", name=f"o{g}")
                            nc.scalar.activation(
                                o_sb[:], ap[:], mybir.ActivationFunctionType.Copy
                            )
                            nc.sync.dma_start(out_v[:, g, :], o_sb[:])
                    if l < 6 and g % 2 == 1:
                        ci = g // 2
                        nc.gpsimd.dma_gather(
                            hT_c[ci][:],
                            h_c[ci][:],
                            tidx_sb[:],
                            num_idxs=256,
                            num_idxs_reg=256,
                            elem_size=NLP,
                            transpose=True,
                            sbuf_tokens_per_rank=128,
                            sbuf_free_dim_per_rank=NLP * 2,
                            queue_num=next_q(),
                        )
                if l < 6:
                    hT_prev = hT_c

    # Align each gather's SWDGE queue with its Tile-assigned DMASW sem lane
    # (ucode locks each DMA sem to one queue; Tile assigns lanes round-robin
    # in scheduled order, so queue must be derived from the lane, not vice
    # versa).
    from concourse.tile_sem_assignment import PROC_NAME_TO_IDX

    lane_to_q = {
        PROC_NAME_TO_IDX[f"DMASW{i}"]: i % NQ for i in range(8)
    }
    for bb in nc.main_func.blocks:
        for inst in bb.instructions:
            if isinstance(inst, mybir.InstDMAGatherAnt):
                proc = getattr(inst, "bass_scheduled_proc", None)
                if proc in lane_to_q:
                    inst.queue_num = lane_to_q[proc]

    nc.compile()
    return nc


def _preprocess(x, edge_index, edge_weight):
    """gcn_norm + node permutation + per-core edge buckets (host side)."""
    ei = np.asarray(edge_index)
    row = np.concatenate([ei[0], np.arange(N)]).astype(np.int64)
    col = np.concatenate([ei[1], np.arange(N)]).astype(np.int64)
    w = np.concatenate(
        [np.asarray(edge_weight, np.float64), np.ones(N, np.float64)]
    )
    deg = np.zeros(N, np.float64)
    np.add.at(deg, col, w)
    dis = np.where(deg > 0, 1.0 / np.sqrt(deg), 0.0)
    norm = (dis[row] * w * dis[col]).astype(np.float32)

    # balance nodes into 80 bins (cap 128 nodes) by in-degree
    indeg = np.bincount(col, minlength=N)
    NB = NCORES * TPC
    order = np.argsort(-indeg, kind="stable")
    load = np.zeros(NB, np.int64)
    cnt = np.zeros(NB, np.int64)
    binof = np.empty(N, np.int64)
    slotof = np.empty(N, np.int64)
    for v in order:
        feas = np.flatnonzero(cnt < 128)
        b = feas[np.argmin(load[feas])]
        binof[v] = b
        slotof[v] = cnt[b]
        cnt[b] += 1
        load[b] += indeg[v]
    assert load.max() <= EPG, f"bin overflow: {load.max()} > {EPG}"
    core = binof // TPC
    grp = binof % TPC
    pid = core * NP_ + grp * 128 + slotof  # permuted global id

    # bucket edges by target bin, assign sequential slots
    ebin = binof[col]
    eorder = np.argsort(ebin, kind="stable")
    ebin_s = ebin[eorder]
    counts = np.bincount(ebin_s, minlength=NB)
    starts = np.concatenate([[0], np.cumsum(counts)[:-1]])
    eslot = np.arange(len(eorder)) - starts[ebin_s]
    ec = ebin_s // TPC
    eg = ebin_s % TPC
    ek = eslot // 128
    ep = eslot % 128
    et = slotof[col[eorder]]
    S = np.zeros((NCORES, 128, TPC, KT, 128), np.float32)
    S[ec, ep, eg, ek, et] = norm[eorder]
    IDX = np.zeros((NCORES, TPC, EPG), np.int64)
    IDX[ec, eg, eslot] = pid[row[eorder]]
    return pid, S, IDX


def kernel(x, edge_index, edge_weight, W1, b1, Wmid, bmid, W7, b7):
    global LAST_EXEC_NS, LAST_TRACE
    trace = os.environ.get("GCN_TRACE") == "1"
    if trace:
        _install_ntff_shim()

    from concourse import bass_utils

    x = np.asarray(x, np.float32)
    pid, S, IDX = _preprocess(x, edge_index, edge_weight)

    # per-core x^T (bf16), empty slots zero
    xT = np.zeros((NCORES, 128, NP_), np.float32)
    xT[pid // NP_, :, pid % NP_] = x
    xT = xT.astype(BF)

    # weights / biases, padded + k-striped, bf16
    def kstripe(W, KL, NL):
        Wp = np.zeros((KL * 128, NL), np.float32)
        Wp[: W.shape[0], : W.shape[1]] = np.asarray(W, np.float32)
        return Wp.reshape(KL, 128, NL).transpose(1, 0, 2).astype(BF)

    Ws = [kstripe(np.asarray(W1), 1, NLP)]
    for i in range(5):
        Ws.append(kstripe(np.asarray(Wmid)[i], 8, NLP))
    Ws.append(kstripe(np.asarray(W7), 8, DOUT))
    bs = []
    for i, b in enumerate([b1] + [np.asarray(bmid)[i] for i in range(5)] + [b7]):
        NL = LAYER_NL[i]
        bp = np.zeros(NL, np.float32)
        bp[: b.shape[0]] = np.asarray(b, np.float32)
        bs.append(np.broadcast_to(bp.astype(BF), (128, NL)).copy())

    ones = np.full((128, 128), 1.0 / 128.0, np.float32).astype(BF)
    tidx = _wrap_idx(np.arange(256))

    if "nc" not in _CACHE:
        _CACHE["nc"] = _build_bass()
    nc = _CACHE["nc"]

    in_maps = []
    for c in range(NCORES):
        eidx_c = np.stack(
            [_wrap_idx(IDX[c, g]) for g in range(TPC)], axis=1
        )  # [128, TPC, 136]
        m = {
            "xT": xT[c],
            "ones": ones,
            "s": np.ascontiguousarray(S[c].astype(BF)),
            "eidx": np.ascontiguousarray(eidx_c),
            "tidx": tidx,
        }
        for l in range(7):
            m[f"w{l}"] = Ws[l]
            m[f"bias{l}"] = bs[l]
        in_maps.append(m)

    res = bass_utils.run_bass_kernel_spmd(
        nc, in_maps, core_ids=list(range(NCORES)), trace=trace
    )
    if trace:
        LAST_EXEC_NS = res.exec_time_ns
        LAST_TRACE = res.profile_json
        print(f"HW exec time: {res.exec_time_ns} ns")
        if res.instructions_and_trace is not None:
            print(f"trace: {res.instructions_and_trace[1]}")

    percore = np.stack([res.results[c]["out"] for c in range(NCORES)])  # [8,1280,256]
    out_full = percore[pid // NP_, pid % NP_]
    return out_full


# revision 16
# speedup vs baseline: 1.5845x; 1.0770x over previous
"""GCN encoder (7-layer GCNConv) on 8 Trainium2 NeuronCores.

Strategy (node-sharded, SPMD):
  - Nodes are permuted and balanced into 8 cores x 10 target-tiles of 128
    slots each (degree-balanced bins so every tile has <= 2176 incoming
    edges = 17 edge-tiles of 128).
  - Per layer l: z = h @ W_l computed locally (dense, bf16 PE matmuls with
    activations as the stationary operand so output is node-major), z is
    AllGathered to every core (bf16, split into two column-halves so the
    second half overlaps the first half's gathers), then per target-tile
    the incoming edge messages are fetched with dma_gather (per-edge row
    gather from the gathered z, 4 SWDGE queues for parallel descriptor
    emission) and segment-summed on the TensorEngine by multiplying with a
    per-tile sparse indicator matrix S (S[e, t] = gcn_norm of edge e into
    target t).  Bias is folded in as one extra matmul; ReLU+cast on the
    Scalar engine.  h -> h^T for the next dense layer is done with
    SBUF-source transposing dma_gathers, chunked per 256 nodes so the next
    layer's dense can start before the whole layer finishes.
  - gcn_norm / edge bucketing / permutation are host-side preprocessing;
    all FLOPs (dense transforms + message aggregation) run on device.
"""

import os
import sys
import types

sys.path.insert(0, "/opt/trn_rl_repo")

import numpy as np
import ml_dtypes

NCORES = 8
N = 10000
E = 160000
DIN = 128
DH = 1000
DOUT = 256

TPC = 10  # target tiles (groups) per core
NP_ = TPC * 128  # 1280 node slots per core
NTOT = NCORES * NP_  # 10240
KT = 17  # edge tiles per group
EPG = KT * 128  # 2176 edge slots per group
NLP = 1024  # padded hidden width
NQ = 4  # SWDGE queues

BF = ml_dtypes.bfloat16

LAYER_NL = [NLP] * 6 + [DOUT]
LAYER_KL = [1] + [8] * 6

_CACHE = {}

LAST_EXEC_NS = None
LAST_TRACE = None


def _install_ntff_shim():
    try:
        import antenv

        if hasattr(antenv, "axon_hooks"):
            return
        from trn_agent_boot.trn_boot import _ntff_profile_via_ctypes

        hook = _ntff_profile_via_ctypes("/opt/axon/libaxon_pjrt.so")
        mod = types.ModuleType("antenv.axon_hooks")
        mod.get_axon_ntff_profile_hook = lambda: hook
        mod.set_axon_ntff_profile_hook = lambda h: None
        sys.modules["antenv.axon_hooks"] = mod
        antenv.axon_hooks = mod
    except Exception:
        pass


def _wrap_idx(idx):
    """[n] int -> [128, n/16] int16 (i -> row i%16, col i//16), 8x replicated."""
    n = idx.shape[0]
    w = np.asarray(idx, np.int16).reshape(n // 16, 16).T
    return np.tile(w, (8, 1))


def _build_bass():
    import concourse.mybir as mybir
    from concourse import bacc, tile

    f32 = mybir.dt.float32
    b16 = mybir.dt.bfloat16
    i16 = mybir.dt.int16
    RG = [list(range(NCORES))]

    nc = bacc.Bacc(
        "TRN2",
        target_bir_lowering=False,
        debug=False,
        num_devices=NCORES,
        num_swdge_queues=NQ,
    )

    xT_d = nc.dram_tensor("xT", [128, NP_], b16, kind="ExternalInput").ap()
    w_d = [
        nc.dram_tensor(
            f"w{l}", [128, LAYER_KL[l], LAYER_NL[l]], b16, kind="ExternalInput"
        ).ap()
        for l in range(7)
    ]
    bias_d = [
        nc.dram_tensor(f"bias{l}", [128, LAYER_NL[l]], b16, kind="ExternalInput").ap()
        for l in range(7)
    ]
    ones_d = nc.dram_tensor("ones", [128, 128], b16, kind="ExternalInput").ap()
    s_d = nc.dram_tensor("s", [128, TPC, KT, 128], b16, kind="ExternalInput").ap()
    eidx_d = nc.dram_tensor(
        "eidx", [128, TPC, EPG // 16], i16, kind="ExternalInput"
    ).ap()
    tidx_d = nc.dram_tensor("tidx", [128, 16], i16, kind="ExternalInput").ap()
    out_d = nc.dram_tensor("out", [NP_, DOUT], f32, kind="ExternalOutput").ap()
    out_v = out_d.rearrange("(g p) f -> p g f", p=128)

    qctr = [0]

    def next_q():
        q = qctr[0] % NQ
        qctr[0] += 1
        return q

    with tile.TileContext(nc) as tc:
        with (
            tc.tile_pool(name="const", bufs=1) as cpool,
            tc.tile_pool(name="w", bufs=2) as wpool,
            tc.tile_pool(name="h", bufs=1) as hpool,
            tc.tile_pool(name="ht", bufs=1) as htpool,
            tc.tile_pool(name="z", bufs=3) as zpool,
            tc.tile_pool(name="m", bufs=2) as mpool,
            tc.tile_pool(name="o", bufs=2) as opool,
            tc.tile_pool(name="psD", bufs=2, space="PSUM") as psD,
            tc.tile_pool(name="psA", bufs=3, space="PSUM") as psA,
            tc.tile_pool(name="dram", bufs=2, space="DRAM") as dpool,
        ):
            xT_sb = cpool.tile([128, NP_], b16)
            nc.sync.dma_start(xT_sb[:], xT_d[:])
            ones_sb = cpool.tile([128, 128], b16)
            nc.sync.dma_start(ones_sb[:], ones_d[:])
            s_sb = cpool.tile([128, TPC, KT, 128], b16)
            nc.sync.dma_start(s_sb[:], s_d[:])
            eidx_sb = cpool.tile([128, TPC, EPG // 16], i16)
            nc.sync.dma_start(eidx_sb[:], eidx_d[:])
            tidx_sb = cpool.tile([128, 16], i16)
            nc.sync.dma_start(tidx_sb[:], tidx_d[:])
            bias_sb = []
            for l in range(7):
                b_sb = cpool.tile([128, LAYER_NL[l]], b16, name=f"bias_sb{l}")
                nc.sync.dma_start(b_sb[:], bias_d[l][:])
                bias_sb.append(b_sb)

            # h / hT in 5 chunks of 256 nodes for fine-grained pipelining
            hT_prev = None
            for l in range(7):
                NL = LAYER_NL[l]
                KL = LAYER_KL[l]
                nch = 2 if NL == NLP else 1
                fcw = 512 if NL == NLP else NL

                w_sb = wpool.tile([128, KL, NL], b16, tag="w", name=f"w_sb{l}")
                nc.sync.dma_start(w_sb[:], w_d[l][:])

                # column-split bounce + gathered buffers (n-th half)
                zbs = [
                    dpool.tile([NP_, fcw], b16, tag=f"zb{n}", name=f"zb{l}_{n}")
                    for n in range(nch)
                ]
                zfs = [
                    dpool.tile(
                        [NTOT, fcw],
                        b16,
                        addr_space="Shared",
                        tag=f"zf{n}",
                        name=f"zf{l}_{n}",
                    )
                    for n in range(nch)
                ]

                # dense: z = h @ W  (node-major out; activations are lhsT)
                for n in range(nch):
                    for m in range(TPC):
                        zp = psD.tile([128, fcw], f32, tag="psD", name=f"zp{l}_{n}_{m}")
                        for k in range(KL):
                            if l == 0:
                                lhsT = xT_sb[:, m * 128 : (m + 1) * 128]
                            else:
                                lhsT = hT_prev[m // 2][
                                    :, k, (m % 2) * 128 : (m % 2) * 128 + 128
                                ]
                            nc.tensor.matmul(
                                zp[:],
                                lhsT,
                                w_sb[:, k, n * 512 : n * 512 + fcw],
                                start=(k == 0),
                                stop=(k == KL - 1),
                            )
                        z_sb = zpool.tile(
                            [128, fcw], b16, tag="z", name=f"z{l}_{n}_{m}"
                        )
                        nc.vector.tensor_copy(z_sb[:], zp[:])
                        nc.sync.dma_start(
                            zbs[n][m * 128 : (m + 1) * 128, :], z_sb[:]
                        )
                    # AllGather this column half as soon as it is complete
                    nc.gpsimd.collective_compute(
                        "AllGather",
                        mybir.AluOpType.bypass,
                        replica_groups=RG,
                        ins=[zbs[n][:].opt()],
                        outs=[zfs[n][:].opt()],
                    )

                if l < 6:
                    h_c = [
                        hpool.tile(
                            [128, 2, NLP], b16, tag=f"h{ci}", name=f"h{l}_c{ci}"
                        )
                        for ci in range(5)
                    ]
                    hT_c = [
                        htpool.tile(
                            [128, 8, 256], b16, tag=f"ht{ci}", name=f"hT{l}_c{ci}"
                        )
                        for ci in range(5)
                    ]

                # aggregation: per target tile, gather messages + S matmuls.
                # n-outer so all fc0 gathers issue before any fc1 gather —
                # fc1 waiting on AG_b must not head-of-line-block fc0 work.
                for n in range(nch):
                    for g in range(TPC):
                        msgs = mpool.tile(
                            [128, KT, fcw], b16, tag="m", name=f"msgs{l}_{g}_{n}"
                        )
                        # HW limit: dma_gather faults above ~1024 idxs/call
                        off = 0
                        while off < EPG:
                            c = min(1024, EPG - off)
                            nc.gpsimd.dma_gather(
                                msgs[:, off // 128 : (off + c) // 128, :],
                                zfs[n][:],
                                eidx_sb[:, g, off // 16 : (off + c) // 16],
                                num_idxs=c,
                                num_idxs_reg=c,
                                elem_size=fcw,
                                elem_step=fcw,
                                queue_num=next_q(),
                            )
                            off += c
                        ap = psA.tile(
                            [128, fcw], f32, tag="psA", name=f"ap{l}_{g}_{n}"
                        )
                        for k in range(KT):
                            nc.tensor.matmul(
                                ap[:],
                                s_sb[:, g, k, :],
                                msgs[:, k, :],
                                start=(k == 0),
                                stop=False,
                            )
                        nc.tensor.matmul(
                            ap[:],
                            ones_sb[:],
                            bias_sb[l][:, n * 512 : n * 512 + fcw],
                            start=False,
                            stop=True,
                        )
                        if l < 6:
                            nc.scalar.activation(
                                h_c[g // 2][:, g % 2, n * 512 : n * 512 + fcw],
                                ap[:],
                                mybir.ActivationFunctionType.Relu,
                            )
                        else:
                            o_sb = opool.tile([128, DOUT], f32, tag="o", name=f"o{g}")
                            nc.scalar.activation(
                                o_sb[:], ap[:], mybir.ActivationFunctionType.Copy
                            )
                            nc.sync.dma_start(out_v[:, g, :], o_sb[:])
                        if l < 6 and n == nch - 1 and g % 2 == 1:
                            ci = g // 2
                            nc.gpsimd.dma_gather(
                                hT_c[ci][:],
                                h_c[ci][:],
                                tidx_sb[:],
                                num_idxs=256,
                                num_idxs_reg=256,
                                elem_size=NLP,
                                transpose=True,
                                sbuf_tokens_per_rank=128,
                                sbuf_free_dim_per_rank=NLP * 2,
                                queue_num=next_q(),
                            )
                if l < 6:
                    hT_prev = hT_c

    # Align each gather's SWDGE queue with its Tile-assigned DMASW sem lane
    # (ucode locks each DMA sem to one queue; Tile assigns lanes round-robin
    # in scheduled order, so queue must be derived from the lane, not vice
    # versa).
    from concourse.tile_sem_assignment import PROC_NAME_TO_IDX

    lane_to_q = {
        PROC_NAME_TO_IDX[f"DMASW{i}"]: i % NQ for i in range(8)
    }
    for bb in nc.main_func.blocks:
        for inst in bb.instructions:
            if isinstance(inst, mybir.InstDMAGatherAnt):
                proc = getattr(inst, "bass_scheduled_proc", None)
                if proc in lane_to_q:
                    inst.queue_num = lane_to_q[proc]

    nc.compile()
    return nc


def _preprocess(x, edge_index, edge_weight):
    """gcn_norm + node permutation + per-core edge buckets (host side)."""
    ei = np.asarray(edge_index)
    row = np.concatenate([ei[0], np.arange(N)]).astype(np.int64)
    col = np.concatenate([ei[1], np.arange(N)]).astype(np.int64)
    w = np.concatenate(
        [np.asarray(edge_weight, np.float64), np.ones(N, np.float64)]
    )
    deg = np.zeros(N, np.float64)
    np.add.at(deg, col, w)
    dis = np.where(deg > 0, 1.0 / np.sqrt(deg), 0.0)
    norm = (dis[row] * w * dis[col]).astype(np.float32)

    # balance nodes into 80 bins (cap 128 nodes) by in-degree
    indeg = np.bincount(col, minlength=N)
    NB = NCORES * TPC
    order = np.argsort(-indeg, kind="stable")
    load = np.zeros(NB, np.int64)
    cnt = np.zeros(NB, np.int64)
    binof = np.empty(N, np.int64)
    slotof = np.empty(N, np.int64)
    for v in order:
        feas = np.flatnonzero(cnt < 128)
        b = feas[np.argmin(load[feas])]
        binof[v] = b
        slotof[v] = cnt[b]
        cnt[b] += 1
        load[b] += indeg[v]
    assert load.max() <= EPG, f"bin overflow: {load.max()} > {EPG}"
    core = binof // TPC
    grp = binof % TPC
    pid = core * NP_ + grp * 128 + slotof  # permuted global id

    # bucket edges by target bin, assign sequential slots
    ebin = binof[col]
    eorder = np.argsort(ebin, kind="stable")
    ebin_s = ebin[eorder]
    counts = np.bincount(ebin_s, minlength=NB)
    starts = np.concatenate([[0], np.cumsum(counts)[:-1]])
    eslot = np.arange(len(eorder)) - starts[ebin_s]
    ec = ebin_s // TPC
    eg = ebin_s % TPC
    ek = eslot // 128
    ep = eslot % 128
    et = slotof[col[eorder]]
    S = np.zeros((NCORES, 128, TPC, KT, 128), np.float32)
    S[ec, ep, eg, ek, et] = norm[eorder]
    IDX = np.zeros((NCORES, TPC, EPG), np.int64)
    IDX[ec, eg, eslot] = pid[row[eorder]]
    return pid, S, IDX


def kernel(x, edge_index, edge_weight, W1, b1, Wmid, bmid, W7, b7):
    global LAST_EXEC_NS, LAST_TRACE
    trace = os.environ.get("GCN_TRACE") == "1"
    if trace:
        _install_ntff_shim()

    from concourse import bass_utils

    x = np.asarray(x, np.float32)
    pid, S, IDX = _preprocess(x, edge_index, edge_weight)

    # per-core x^T (bf16), empty slots zero
    xT = np.zeros((NCORES, 128, NP_), np.float32)
    xT[pid // NP_, :, pid % NP_] = x
    xT = xT.astype(BF)

    # weights / biases, padded + k-striped, bf16
    def kstripe(W, KL, NL):
        Wp = np.zeros((KL * 128, NL), np.float32)
        Wp[: W.shape[0], : W.shape[1]] = np.asarray(W, np.float32)
        return Wp.reshape(KL, 128, NL).transpose(1, 0, 2).astype(BF)

    Ws = [kstripe(np.asarray(W1), 1, NLP)]
    for i in range(5):
        Ws.append(kstripe(np.asarray(Wmid)[i], 8, NLP))
    Ws.append(kstripe(np.asarray(W7), 8, DOUT))
    bs = []
    for i, b in enumerate([b1] + [np.asarray(bmid)[i] for i in range(5)] + [b7]):
        NL = LAYER_NL[i]
        bp = np.zeros(NL, np.float32)
        bp[: b.shape[0]] = np.asarray(b, np.float32)
        bs.append(np.broadcast_to(bp.astype(BF), (128, NL)).copy())

    ones = np.full((128, 128), 1.0 / 128.0, np.float32).astype(BF)
    tidx = _wrap_idx(np.arange(256))

    if "nc" not in _CACHE:
        _CACHE["nc"] = _build_bass()
    nc = _CACHE["nc"]

    in_maps = []
    for c in range(NCORES):
        eidx_c = np.stack(
            [_wrap_idx(IDX[c, g]) for g in range(TPC)], axis=1
        )  # [128, TPC, 136]
        m = {
            "xT": xT[c],
            "ones": ones,
            "s": np.ascontiguousarray(S[c].astype(BF)),
            "eidx": np.ascontiguousarray(eidx_c),
            "tidx": tidx,
        }
        for l in range(7):
            m[f"w{l}"] = Ws[l]
            m[f"bias{l}"] = bs[l]
        in_maps.append(m)

    res = bass_utils.run_bass_kernel_spmd(
        nc, in_maps, core_ids=list(range(NCORES)), trace=trace
    )
    if trace:
        LAST_EXEC_NS = res.exec_time_ns
        LAST_TRACE = res.profile_json
        print(f"HW exec time: {res.exec_time_ns} ns")
        if res.instructions_and_trace is not None:
            print(f"trace: {res.instructions_and_trace[1]}")

    percore = np.stack([res.results[c]["out"] for c in range(NCORES)])  # [8,1280,256]
    out_full = percore[pid // NP_, pid % NP_]
    return out_full
